# revision 33
# baseline (speedup 1.0000x reference)
"""GAT (3-layer, 8-head) message-passing kernel for one TRN2 chip (8 NeuronCores).

Strategy (dst-sharded, CSR "node-per-partition / edge-per-column" layout):
  - Nodes sharded 6250/core and assigned *positions* 0..6655 per core.  Each
    128-position chunk holds nodes of similar in-degree (2D snake sort on
    per-class degrees), so the per-chunk max degree C_k is close to the mean.
    Positions 0 and PHALF are dummy rows whose alpha_src is set to -60000;
    padded gather slots point there and contribute exp(-big) = 0.
  - Per layer every core computes the dense part (h = act @ W, alpha_s/d) for
    its own positions, transposes it into a packed fp16 table row
    [h(64) | a_s(8) | pad | a_d(8)@96 | pad], and the two position-halves are
    AllGather-ed so every core holds the full table (tblA/tblB, int16-indexable).
  - Edges grouped by dst: chunk k's class-X strip gathers [128, C_k, 128] rows
    (edge c of node p lands on partition p; chunks with C>24 split over two
    SWDGE queues).  Chunks are batched into super-strips of uniform stride Cg
    so z/leaky/exp/mult/reduce are one DVE/ACT op each; garbage columns keep
    alpha_s = -60000 (restored post-reduce on the scalar engine) so they
    reduce to zero.  alpha_dst is a per-partition broadcast from the
    position-major a_d kept on-chip.  A DVE free-dim reduce accumulates
    numerator+denominator into a persistent SBUF accumulator -- no edge-phase
    matmuls, no DRAM partials, no compaction gathers.  Next-layer dense/
    table/AllGather emission is interleaved into the B-phase so the layer
    boundary hides under the gather drains.
  - Final: matmul-pooling per graph -> [64,64] partials per core; host sums
    partials and applies the tiny linear head + log_softmax.
"""

import math
import os

import numpy as np

# ---------------- problem constants (hardcoded) ----------------
N = 50000
E = 1600000
F_IN = 160
H = 8
C = 8
HC = 64
G = 64
NEG_SLOPE = 0.2

NCORES = 8
NSHARD = N // NCORES          # 6250
NHALF0 = (NSHARD + 1) // 2    # 3125 nodes per shard half (class split by n)
NSP = 6656                    # padded positions = 52*128 = 13*512
NCHUNK = NSP // 128           # 52
PHALF = NSP // 2              # 3328 positions per half
TBL_ROWS = NCORES * NSP       # 53248
HTBL = NCORES * PHALF         # 26624 rows per half-table (< 32768 => int16)

EXP_SHIFT = -10.0
MASK_NEG = -60000.0


# ================= host-side structure building =================

def _snake_order(dA, dB, block=512):
    """Order node ids so consecutive runs have similar (dA, dB).

    Primary: dA descending.  Within blocks of `block`, re-sort by dB
    descending, so a 128-node chunk has a tight range in BOTH degrees.
    """
    o1 = np.argsort(-dA, kind='stable')
    out = []
    for b in range(0, len(o1), block):
        blk = o1[b:b + block]
        out.append(blk[np.argsort(-dB[blk], kind='stable')])
    return np.concatenate(out) if out else o1


SPLIT_K = 28  # part boundary: 28*128 = 3584 = 7*512 (dense-chunk aligned)


def _make_groups(Cs, cap=96, kkmax=8, brk=SPLIT_K):
    """Group consecutive chunks into super-strips: (k0, KK, Cg) with
    KK*Cg <= cap, Cg = max C over the group (uniform DVE stride).
    Never spans the `brk` chunk boundary (interleaved emission point)."""
    groups = []
    i = 0
    while i < NCHUNK:
        mx = Cs[i]
        kk = 1
        while (i + kk < NCHUNK and kk < kkmax
               and (kk + 1) * max(mx, Cs[i + kk]) <= cap
               and not (i < brk <= i + kk)):
            mx = max(mx, Cs[i + kk])
            kk += 1
        groups.append((i, kk, mx))
        i += kk
    return groups


def _host_preprocess(edge_index, batch):
    src = np.asarray(edge_index[0], np.int64)
    dst = np.asarray(edge_index[1], np.int64)
    loops = np.arange(N, dtype=np.int64)
    src = np.concatenate([src, loops])
    dst = np.concatenate([dst, loops])

    order = np.argsort(dst, kind='stable')
    src_s, dst_s = src[order], dst[order]
    bounds = np.searchsorted(dst_s, np.arange(N + 1))

    s_n = src_s % NSHARD
    s_clsA = s_n < NHALF0                      # class of each edge (by src)

    # per-node class degrees (for position sorting)
    dA = np.bincount(dst_s[s_clsA], minlength=N)
    dB = np.bincount(dst_s[~s_clsA], minlength=N)

    # ---- pass 1: position maps.  Nodes are re-assigned to cores so that
    # all 8 cores see near-identical per-chunk degree profiles: per class,
    # sort ALL nodes of that class by (dA, dB) snake order, then deal
    # consecutive 128-node chunks round-robin to the cores. ----
    posmap = np.empty(N, np.int64)       # node -> position on its core
    coremap = np.empty(N, np.int64)      # node -> owning core
    for half in range(2):
        cls_nodes = np.concatenate([
            np.arange(c_ * NSHARD + half * NHALF0,
                      min(c_ * NSHARD + half * NHALF0 + NHALF0,
                          (c_ + 1) * NSHARD))
            for c_ in range(NCORES)])
        ordh = _snake_order(dA[cls_nodes], dB[cls_nodes], block=4096)
        snodes = cls_nodes[ordh]
        # deal: global chunk g (128 slots incl per-core dummies) ->
        # core g % 8, local chunk g // 8.  Slot 0 of each core's chunk 0
        # is the dummy; account by dealing positions 1.. within each core.
        for c_ in range(NCORES):
            share = snodes[c_::NCORES]
            posmap[share] = half * PHALF + 1 + np.arange(len(share))
            coremap[share] = c_

    # src table row: core * PHALF + (position within half)
    s_core = coremap[src_s]
    s_row = s_core * PHALF + (posmap[src_s] % PHALF)

    # ---- pass 2: per-core per-chunk max degree, shared across cores ----
    CA = np.zeros((NCORES, NCHUNK), np.int64)
    CB = np.zeros((NCORES, NCHUNK), np.int64)
    node_pos = posmap  # global node -> local position
    degA_all = np.bincount(dst_s[s_clsA], minlength=N)
    degB_all = np.bincount(dst_s[~s_clsA], minlength=N)
    for c_ in range(NCORES):
        nodes = np.where(coremap == c_)[0]
        ch = node_pos[nodes] // 128
        np.maximum.at(CA[c_], ch, degA_all[nodes])
        np.maximum.at(CB[c_], ch, degB_all[nodes])
    CAs = np.maximum(CA.max(axis=0), 1).astype(np.int64)
    CBs = np.maximum(CB.max(axis=0), 1).astype(np.int64)

    # ---- pass 3: per-core idx arrays ----
    def wrap(arr):  # [M] int16 -> [128, M//16] (16-part wrap, replicated x8)
        m = arr.shape[0]
        w = arr.reshape(m // 16, 16).T
        return np.tile(w, (8, 1))

    idx_cores = []
    for c_ in range(NCORES):
        pad_row = c_ * PHALF  # dummy position row (alpha_s = -60000)
        strips = []
        for cls, Cs in ((0, CAs), (1, CBs)):
            mats = [np.full((Cs[k], 128), pad_row, np.int64)
                    for k in range(NCHUNK)]
            strips.append(mats)
        for g in np.where(coremap == c_)[0]:
            sl = slice(bounds[g], bounds[g + 1])
            rows = s_row[sl]
            cls = s_clsA[sl]
            pos = node_pos[g]
            k, p = pos // 128, pos % 128
            ra = rows[cls]
            rb = rows[~cls]
            strips[0][k][0:len(ra), p] = ra
            strips[1][k][0:len(rb), p] = rb
        parts = []
        for cls in range(2):
            for k in range(NCHUNK):
                parts.append(wrap(strips[cls][k].reshape(-1)))
        idx_cores.append(np.concatenate(parts, axis=1).astype(np.int16))

    # strip offsets in int16 units within idx row
    offs = []
    o = 0
    for Cs in (CAs, CBs):
        oo = []
        for k in range(NCHUNK):
            oo.append(o)
            o += 8 * int(Cs[k])
        offs.append(oo)

    batch = np.asarray(batch, np.int64)
    cnt = np.bincount(batch, minlength=G).astype(np.float32)

    p01s = []
    for c_ in range(NCORES):
        nodes = np.where(coremap == c_)[0]
        pos = node_pos[nodes]
        p = np.zeros((NCHUNK, 128, G), np.float16)
        p[pos // 128, pos % 128, batch[nodes]] = 1.0
        p01s.append(p)

    grpA = _make_groups(CAs.tolist())
    grpB = _make_groups(CBs.tolist())

    pos_of_node = node_pos
    return (idx_cores, p01s, cnt, CAs.tolist(), CBs.tolist(),
            offs[0], offs[1], o, pos_of_node, coremap, grpA, grpB)


# ================= bass program =================

def _build_bass(CA, CB, offA, offB, idxw, grpA, grpB):
    import concourse.bass as bass
    import concourse.mybir as mybir
    import concourse.tile as tile
    from concourse import bacc
    from concourse.masks import make_identity

    fp16 = mybir.dt.float16
    fp32 = mybir.dt.float32
    i16 = mybir.dt.int16
    AF = mybir.ActivationFunctionType
    ALU = mybir.AluOpType
    AX = mybir.AxisListType

    KLAYERS = int(os.environ.get('KLAYERS', '3'))
    KCOLL = os.environ.get('KCOLL', '1') == '1'
    KEDGE = int(os.environ.get('KEDGE', '9'))
    MMAX = max(kk * cg for _, kk, cg in grpA + grpB)
    KKMAX = max(kk for _, kk, _ in grpA + grpB)

    nc = bacc.Bacc(num_swdge_queues=4)

    # ---- I/O ----
    xT = nc.dram_tensor("xT", [F_IN, NSP], fp16, kind="ExternalInput")
    idxs = nc.dram_tensor("idxs", [128, idxw], i16, kind="ExternalInput")
    p01 = nc.dram_tensor("p01", [NCHUNK, 128, G], fp16, kind="ExternalInput")
    Ws = nc.dram_tensor("Ws", [4, 128, HC], fp16, kind="ExternalInput")
    As = nc.dram_tensor("As", [6, HC, H], fp16, kind="ExternalInput")
    brep = nc.dram_tensor("brep", [3, 128, HC], fp32, kind="ExternalInput")
    out_d = nc.dram_tensor("out", [G, HC], fp32, kind="ExternalOutput")

    # ---- internal DRAM ----
    tbl_shard = nc.dram_tensor("tbl_shard", [NSP, 128], fp16)
    tblA = nc.dram_tensor("tblA", [HTBL, 128], fp16, addr_space="Shared")
    tblB = nc.dram_tensor("tblB", [HTBL, 128], fp16, addr_space="Shared")

    with tile.TileContext(nc) as tc:
        with (
            tc.tile_pool(name="const", bufs=1) as constp,
            tc.tile_pool(name="pers", bufs=1) as pers,
            tc.tile_pool(name="work", bufs=1) as work,
            tc.tile_pool(name="edge", bufs=4) as edgep,
            tc.tile_pool(name="edges", bufs=4) as edgesm,
            tc.tile_pool(name="edgetn", bufs=2) as edgetn,
            tc.tile_pool(name="dps", bufs=2, space="PSUM") as psd,
        ):
            cvals = set()
            for cc in set(CA) | set(CB):
                if cc <= 24:
                    cvals.add(cc)
                else:
                    cvals.add(cc // 2)
                    cvals.add(cc - cc // 2)
            regs = {cc: nc.gpsimd.to_reg(128 * cc) for cc in sorted(cvals)}
            expb = constp.tile([128, 1], fp32)
            nc.vector.memset(expb[:], EXP_SHIFT)
            negc = constp.tile([128, 1], fp16)
            nc.vector.memset(negc[:], MASK_NEG)
            ident = constp.tile([128, 128], fp16)
            make_identity(nc, ident[:])
            ws_t = constp.tile([128, 4 * HC], fp16)
            nc.sync.dma_start(ws_t[:].rearrange("p (a h) -> p a h", a=4),
                              Ws[:].rearrange("a p h -> p a h"))
            as_t = constp.tile([HC, 6 * H], fp16)
            nc.sync.dma_start(as_t[:].rearrange("c (s h) -> c s h", s=6),
                              As[:].rearrange("s c h -> c s h"))
            brep_t = constp.tile([128, 3 * HC], fp32)
            nc.sync.dma_start(brep_t[:].rearrange("p (l h) -> p l h", l=3),
                              brep[:].rearrange("l p h -> p l h"))

            act_fm = pers.tile([128, NSP], fp16)
            act_fmb = pers.tile([F_IN - 128, NSP], fp16)
            nc.sync.dma_start(act_fm[:], xT[0:128, :])
            nc.sync.dma_start(act_fmb[:], xT[128:F_IN, :])
            act_nm = pers.tile([128, NCHUNK, HC], fp16)
            adpos = pers.tile([128, NCHUNK, H], fp16)
            cmp = pers.tile([128, NCHUNK, 72], fp32)

            qrr = [0]
            gcount = [0]
            KTRED = os.environ.get('KTRED', '0') == '1'

            def reduce_kc(out_ap, in_kcf):
                """out[p,k,f] = sum_c in[p,k,c,f] via HW transpose-reduce
                (contiguous read), falling back to the strided rearrange."""
                ve = nc.vector
                if not KTRED:
                    p, kk, cg, f = in_kcf.shape
                    return ve.tensor_reduce(
                        out_ap, in_kcf.rearrange("p k c f -> p k f c"),
                        axis=AX.X, op=ALU.add)
                return ve.add_instruction(
                    mybir.InstTensorReduce(
                        name=f"I-{ve.bass.next_id()}",
                        op=ALU.add, axis=AX.X,
                        ins=[ve.lower_ap(in_kcf, opt=False)],
                        outs=[ve.lower_ap(out_ap)],
                        apply_absolute_value=None,
                        apply_transpose=True,
                        negate=None))

            def emit_dense(layer, c5lo, c5hi, trbuf):
                for ch in range(c5lo, c5hi):
                    cs512 = slice(ch * 512, (ch + 1) * 512)
                    hp = psd.tile([HC, 512], fp32, tag="dps")
                    if layer == 0:
                        nc.tensor.matmul(hp[:], ws_t[:, 0:HC],
                                         act_fm[0:128, cs512],
                                         start=True, stop=False)
                        nc.tensor.matmul(hp[:], ws_t[0:F_IN - 128, HC:2 * HC],
                                         act_fmb[:, cs512],
                                         start=False, stop=True)
                    else:
                        nc.tensor.matmul(
                            hp[:], ws_t[0:64, (layer + 1) * HC:(layer + 2) * HC],
                            act_fm[0:64, cs512], start=True, stop=True)
                    nc.scalar.activation(trbuf[0:64, cs512], hp[:], AF.Copy)
                    ap_ = psd.tile([40, 512], fp32, tag="dps")
                    nc.tensor.matmul(
                        ap_[0:H, :], as_t[:, 2 * layer * H:(2 * layer + 1) * H],
                        trbuf[0:64, cs512], start=True, stop=True)
                    nc.tensor.matmul(
                        ap_[32:32 + H, :],
                        as_t[:, (2 * layer + 1) * H:(2 * layer + 2) * H],
                        trbuf[0:64, cs512], start=True, stop=True,
                        tile_position=(0, 32))
                    nc.scalar.activation(trbuf[64:72, cs512], ap_[0:H, :],
                                         AF.Copy)
                    nc.scalar.activation(trbuf[96:96 + H, cs512],
                                         ap_[32:32 + H, :], AF.Copy)
                if c5lo == 0:
                    # dummy/pad rows: alpha_src = -60000 -> ex == 0
                    nc.vector.memset(trbuf[64:72, 0:1], MASK_NEG)
                    nc.vector.memset(trbuf[64:72, PHALF:PHALF + 1], MASK_NEG)

            def emit_table(klo, khi, trbuf):
                for ch in range(klo, khi):
                    tp = psd.tile([128, 104], fp16, tag="dps")
                    nc.tensor.matmul(tp[:], trbuf[0:104, ch * 128:(ch + 1) * 128],
                                     ident[0:104, 0:104], is_transpose=True,
                                     start=True, stop=True)
                    tabst = work.tile([128, 104], fp16, tag="tabst")
                    nc.scalar.activation(tabst[:], tp[:], AF.Copy)
                    nc.scalar.activation(adpos[:, ch, :], tabst[:, 96:104],
                                         AF.Copy)
                    # cols 104:128 of each table row stay garbage -- consumers
                    # only read cols 0:72.  Scalar-engine DMA queue: keeps
                    # these (dep-pending) writes off the sync ring that the
                    # edge-phase idx loads ride on.
                    nc.scalar.dma_start(
                        tbl_shard[ch * 128:(ch + 1) * 128, 0:104], tabst[:])

            def emit_ag(half):
                if not KCOLL:
                    return
                if half == 0:
                    nc.gpsimd.collective_compute(
                        "AllGather", mybir.AluOpType.bypass,
                        ins=[tbl_shard[0:PHALF, :]], outs=[tblA[:]],
                        replica_groups=[list(range(NCORES))])
                else:
                    nc.gpsimd.collective_compute(
                        "AllGather", mybir.AluOpType.bypass,
                        ins=[tbl_shard[PHALF:NSP, :]], outs=[tblB[:]],
                        replica_groups=[list(range(NCORES))])

            def emit_group(layer, phase, CL, offL, tbl, k0, KK, Cg):
                M = KK * Cg
                g1t = edgep.tile([128, MMAX, 128], fp16, tag="g1")
                if layer == 0 and gcount[0] < 4:
                    # first rotations: clear whatever (possibly NaN) bytes
                    # SBUF held; alpha_s = -60000 so unwritten cols -> ex == 0
                    nc.vector.memset(g1t[:], MASK_NEG)
                gcount[0] += 1
                o0 = offL[k0]
                ow = offL[k0 + KK - 1] + 8 * CL[k0 + KK - 1] - o0
                idxt = edgesm.tile([128, 8 * MMAX], i16, tag="idx")
                nc.sync.dma_start(idxt[:, 0:ow], idxs[:, o0:o0 + ow])
                for j in range(KK):
                    k = k0 + j
                    Ck = CL[k]
                    # split big chunks across two queues to break up
                    # long single-queue drain poles
                    parts = ((0, Ck),) if Ck <= 24 else \
                        ((0, Ck // 2), (Ck // 2, Ck - Ck // 2))
                    for (cb, cn) in parts:
                        nc.gpsimd.dma_gather(
                            g1t[:, j * Cg + cb:j * Cg + cb + cn, :], tbl[:],
                            idxt[:, offL[k] - o0 + 8 * cb:
                                 offL[k] - o0 + 8 * (cb + cn)],
                            num_idxs=128 * cn, num_idxs_reg=regs[cn],
                            elem_size=128, single_packet=False,
                            queue_num=qrr[0] % 4)
                        qrr[0] += 1
                zt = edgesm.tile([128, MMAX, H], fp16, tag="z")
                nc.vector.tensor_tensor(
                    zt[:, 0:M, :].rearrange("p (k c) h -> p k c h", k=KK),
                    g1t[:, 0:M, 64:72].rearrange("p (k c) h -> p k c h", k=KK),
                    adpos[:, k0:k0 + KK, :].unsqueeze(2)
                    .to_broadcast([128, KK, Cg, H]),
                    op=ALU.add)
                nc.vector.scalar_tensor_tensor(
                    zt[:, 0:M, :], zt[:, 0:M, :], NEG_SLOPE,
                    zt[:, 0:M, :], op0=ALU.mult, op1=ALU.max)
                nc.scalar.activation(g1t[:, 0:M, 64:72], zt[:, 0:M, :],
                                     AF.Exp, bias=expb[:], scale=1.0)
                nc.vector.tensor_tensor(
                    g1t[:, 0:M, 0:64].rearrange("p c (h j) -> p c h j", h=H),
                    g1t[:, 0:M, 0:64].rearrange("p c (h j) -> p c h j", h=H),
                    g1t[:, 0:M, 64:72].unsqueeze(3)
                    .to_broadcast([128, M, H, C]),
                    op=ALU.mult)
                if phase == 0:
                    reduce_kc(cmp[:, k0:k0 + KK, :],
                              g1t[:, 0:M, 0:72].rearrange(
                                  "p (k c) f -> p k c f", k=KK))
                else:
                    tn = edgetn.tile([128, KKMAX, 72], fp32, tag="tn")
                    reduce_kc(tn[:, 0:KK, :],
                              g1t[:, 0:M, 0:72].rearrange(
                                  "p (k c) f -> p k c f", k=KK))
                    nc.vector.tensor_tensor(
                        cmp[:, k0:k0 + KK, :], cmp[:, k0:k0 + KK, :],
                        tn[:, 0:KK, :], op=ALU.add)
                nc.scalar.activation(
                    g1t[:, :, 64:72],
                    negc[:].unsqueeze(2).to_broadcast([128, MMAX, H]),
                    AF.Copy)

            def emit_epilogue(layer, klo, khi):
                nk = khi - klo
                rs = work.tile([128, 32, H], fp32, tag="rs")
                nc.vector.tensor_scalar_max(cmp[:, klo:khi, 64:72],
                                            cmp[:, klo:khi, 64:72], 1e-30)
                nc.vector.reciprocal(rs[:, 0:nk, :], cmp[:, klo:khi, 64:72])
                ov = work.tile([128, 32, HC], fp32, tag="ov")
                nc.vector.tensor_tensor(
                    ov[:, 0:nk, :].rearrange("p c (h j) -> p c h j", h=H),
                    cmp[:, klo:khi, 0:64].rearrange(
                        "p c (h j) -> p c h j", h=H),
                    rs[:, 0:nk, :].unsqueeze(3)
                    .to_broadcast([128, nk, H, C]),
                    op=ALU.mult)
                nc.vector.tensor_tensor(
                    ov[:, 0:nk, :], ov[:, 0:nk, :],
                    (brep_t[:, layer * HC:(layer + 1) * HC]
                     .unsqueeze(1).to_broadcast([128, nk, HC])),
                    op=ALU.add)
                mneg = cmp[:, klo:khi, 0:64]  # num consumed; reuse as scratch
                nc.vector.tensor_scalar_min(mneg, ov[:, 0:nk, :], 0.0)
                nc.scalar.activation(mneg, mneg, AF.Exp)
                nc.vector.tensor_scalar_max(ov[:, 0:nk, :], ov[:, 0:nk, :], 0.0)
                nc.vector.scalar_tensor_tensor(
                    ov[:, 0:nk, :], mneg, -1.0, ov[:, 0:nk, :],
                    op0=ALU.add, op1=ALU.add)
                nc.scalar.activation(act_nm[:, klo:khi, :], ov[:, 0:nk, :],
                                     AF.Copy)

            def emit_transposes(klo, khi):
                for ch in range(klo, khi):
                    tp2 = psd.tile([64, 128], fp16, tag="dps")
                    nc.tensor.matmul(tp2[:], act_nm[:, ch, :],
                                     ident[:, 0:128], is_transpose=True,
                                     start=True, stop=True)
                    nc.scalar.activation(
                        act_fm[0:64, ch * 128:(ch + 1) * 128], tp2[:],
                        AF.Copy)

            SPL = 28  # == SPLIT_K (group lists never span it)
            DSPL = SPL * 128 // 512  # dense 512-chunk at the split

            # layer 0 prologue: dense + table + AGs
            trbuf = work.tile([128, NSP], fp16, tag="trbuf")
            emit_dense(0, 0, NSP // 512, trbuf)
            emit_table(0, PHALF // 128, trbuf)
            emit_ag(0)
            emit_table(PHALF // 128, NCHUNK, trbuf)
            emit_ag(1)

            for layer in range(KLAYERS):
                last = layer == KLAYERS - 1
                for (k0, KK, Cg) in grpA:
                    emit_group(layer, 0, CA, offA, tblA, k0, KK, Cg)
                for (k0, KK, Cg) in grpB:
                    if k0 >= SPL:
                        break
                    emit_group(layer, 1, CB, offB, tblB, k0, KK, Cg)
                emit_epilogue(layer, 0, SPL)
                if not last:
                    emit_transposes(0, SPL)
                    trbuf = work.tile([128, NSP], fp16, tag="trbuf")
                # stagger next-layer dense/table emission between the first
                # B-part2 groups so no engine queue gets one big head-of-line
                # block ahead of the edge-phase consumer chains
                p2 = [g for g in grpB if g[0] >= SPL]
                for i, (k0, KK, Cg) in enumerate(p2):
                    emit_group(layer, 1, CB, offB, tblB, k0, KK, Cg)
                    if not last and i == 0:
                        emit_dense(layer + 1, 0, DSPL, trbuf)
                    if not last and i == 1:
                        emit_table(0, PHALF // 128, trbuf)
                if not last and len(p2) < 2:
                    emit_dense(layer + 1, 0, DSPL, trbuf)
                    emit_table(0, PHALF // 128, trbuf)
                if not last:
                    emit_ag(0)
                emit_epilogue(layer, SPL, NCHUNK)
                if not last:
                    emit_transposes(SPL, NCHUNK)
                    emit_dense(layer + 1, DSPL, NSP // 512, trbuf)
                    emit_table(PHALF // 128, NCHUNK, trbuf)
                    emit_ag(1)

            # ---------- pooling ----------
            p01t = work.tile([128, NCHUNK, G], fp16, tag="trbuf")
            nc.sync.dma_start(p01t[:], p01[:].rearrange("c p g -> p c g"))
            poolp = psd.tile([G, HC], fp32, tag="dps")
            for ch in range(NCHUNK):
                nc.tensor.matmul(poolp[:], p01t[:, ch, :], act_nm[:, ch, :],
                                 start=(ch == 0), stop=(ch == NCHUNK - 1))
            outsb = pers.tile([G, HC], fp32)
            nc.vector.tensor_copy(outsb[:], poolp[:])
            nc.sync.dma_start(out_d[:], outsb[:])

    nc.finalize()
    return nc


# ================= entry point =================

def _host_preprocess_cached(edge_index, batch):
    """Cache the (slow, pure-function-of-inputs) host preprocessing."""
    import hashlib
    import pickle
    key = hashlib.sha256()
    key.update(edge_index.tobytes())
    key.update(batch.tobytes())
    key.update(f"v12csr:{N}:{E}:{NSP}".encode())
    path = f"/tmp/gat_pre_{key.hexdigest()[:16]}.pkl"
    try:
        with open(path, "rb") as f:
            return pickle.load(f)
    except Exception:
        pass
    res = _host_preprocess(edge_index, batch)
    try:
        with open(path + ".tmp", "wb") as f:
            pickle.dump(res, f, protocol=4)
        os.replace(path + ".tmp", path)
    except Exception:
        pass
    return res


def kernel(x, edge_index, batch, W1, a1s, a1d, b1, W2, a2s, a2d, b2,
           W3, a3s, a3d, b3, Wlin, blin):
    x = np.asarray(x, np.float32)
    (idx_cores, p01s, cnt, CA, CB, offA, offB, idxw,
     pos_of_node, coremap, grpA, grpB) = _host_preprocess_cached(
        np.asarray(edge_index), np.asarray(batch))

    def amat(a):  # [H, C] -> [HC, H] block-diagonal
        m = np.zeros((HC, H), np.float16)
        a = np.asarray(a, np.float16)
        for h_ in range(H):
            m[h_ * C:(h_ + 1) * C, h_] = a[h_]
        return m

    Ws = np.zeros((4, 128, HC), np.float16)
    Ws[0] = np.asarray(W1, np.float16)[0:128]
    Ws[1, 0:F_IN - 128] = np.asarray(W1, np.float16)[128:F_IN]
    Ws[2, 0:HC] = np.asarray(W2, np.float16)
    Ws[3, 0:HC] = np.asarray(W3, np.float16)
    As = np.stack([amat(a1s), amat(a1d), amat(a2s), amat(a2d),
                   amat(a3s), amat(a3d)])
    brep = np.stack([np.tile(np.asarray(b, np.float32)[None, :], (128, 1))
                     for b in (b1, b2, b3)])

    in_maps = []
    for c_ in range(NCORES):
        nodes = np.where(coremap == c_)[0]
        pos = pos_of_node[nodes]
        xTa = np.zeros((F_IN, NSP), np.float16)
        xTa[:, pos] = x[nodes].T
        in_maps.append({
            "xT": xTa, "idxs": idx_cores[c_], "p01": p01s[c_],
            "Ws": Ws, "As": As, "brep": brep,
        })

    nc = _build_bass(CA, CB, offA, offB, idxw, grpA, grpB)
    from concourse.bass_utils import run_bass_kernel_spmd
    res = run_bass_kernel_spmd(nc, in_maps, list(range(NCORES)))
    global LAST_RESULT
    LAST_RESULT = res

    pooled = np.zeros((G, HC), np.float64)
    for r in res.results:
        pooled += r["out"].astype(np.float64)
    pooled = (pooled / np.maximum(cnt, 1.0)[:, None]).astype(np.float32)
    logits = (pooled @ np.asarray(Wlin, np.float32)
              + np.asarray(blin, np.float32))
    m = logits.max(axis=1, keepdims=True)
    lse = np.log(np.exp(logits - m).sum(axis=1, keepdims=True)) + m
    return (logits - lse).astype(np.float32)


# revision 35
# speedup vs baseline: 1.0138x; 1.0138x over previous
"""GAT (3-layer, 8-head) message-passing kernel for one TRN2 chip (8 NeuronCores).

Strategy (dst-sharded, CSR "node-per-partition / edge-per-column" layout):
  - Nodes sharded 6250/core and assigned *positions* 0..6655 per core.  Each
    128-position chunk holds nodes of similar in-degree (2D snake sort on
    per-class degrees), so the per-chunk max degree C_k is close to the mean.
    Positions 0 and PHALF are dummy rows whose alpha_src is set to -60000;
    padded gather slots point there and contribute exp(-big) = 0.
  - Per layer every core computes the dense part (h = act @ W, alpha_s/d) for
    its own positions, transposes it into a packed fp16 table row
    [h(64) | a_s(8) | pad | a_d(8)@96 | pad], and the two position-halves are
    AllGather-ed so every core holds the full table (tblA/tblB, int16-indexable).
  - Edges grouped by dst: chunk k's class-X strip gathers [128, C_k, 128] rows
    (edge c of node p lands on partition p; chunks with C>24 split over two
    SWDGE queues).  Chunks are batched into super-strips of uniform stride Cg
    so z/leaky/exp/mult/reduce are one DVE/ACT op each; garbage columns keep
    alpha_s = -60000 (restored post-reduce on the scalar engine) so they
    reduce to zero.  alpha_dst is a per-partition broadcast from the
    position-major a_d kept on-chip.  A DVE free-dim reduce accumulates
    numerator+denominator into a persistent SBUF accumulator -- no edge-phase
    matmuls, no DRAM partials, no compaction gathers.  Next-layer dense/
    table/AllGather emission is interleaved into the B-phase so the layer
    boundary hides under the gather drains.
  - Final: matmul-pooling per graph -> [64,64] partials per core; host sums
    partials and applies the tiny linear head + log_softmax.
"""

import math
import os

import numpy as np

# ---------------- problem constants (hardcoded) ----------------
N = 50000
E = 1600000
F_IN = 160
H = 8
C = 8
HC = 64
G = 64
NEG_SLOPE = 0.2

NCORES = 8
NSHARD = N // NCORES          # 6250
NHALF0 = (NSHARD + 1) // 2    # 3125 nodes per shard half (class split by n)
NSP = 6656                    # padded positions = 52*128 = 13*512
NCHUNK = NSP // 128           # 52
PHALF = NSP // 2              # 3328 positions per half
TBL_ROWS = NCORES * NSP       # 53248
HTBL = NCORES * PHALF         # 26624 rows per half-table (< 32768 => int16)

EXP_SHIFT = -10.0
MASK_NEG = -60000.0


# ================= host-side structure building =================

def _snake_order(dA, dB, block=512):
    """Order node ids so consecutive runs have similar (dA, dB).

    Primary: dA descending.  Within blocks of `block`, re-sort by dB
    descending, so a 128-node chunk has a tight range in BOTH degrees.
    """
    o1 = np.argsort(-dA, kind='stable')
    out = []
    for b in range(0, len(o1), block):
        blk = o1[b:b + block]
        out.append(blk[np.argsort(-dB[blk], kind='stable')])
    return np.concatenate(out) if out else o1


SPLIT_K = 28  # part boundary: 28*128 = 3584 = 7*512 (dense-chunk aligned)


def _make_groups(Cs, cap=96, kkmax=8, brk=SPLIT_K):
    """Group consecutive chunks into super-strips: (k0, KK, Cg) with
    KK*Cg <= cap, Cg = max C over the group (uniform DVE stride).
    Never spans the `brk` chunk boundary (interleaved emission point)."""
    groups = []
    i = 0
    while i < NCHUNK:
        mx = Cs[i]
        kk = 1
        while (i + kk < NCHUNK and kk < kkmax
               and (kk + 1) * max(mx, Cs[i + kk]) <= cap
               and not (i < brk <= i + kk)):
            mx = max(mx, Cs[i + kk])
            kk += 1
        groups.append((i, kk, mx))
        i += kk
    return groups


def _host_preprocess(edge_index, batch):
    src = np.asarray(edge_index[0], np.int64)
    dst = np.asarray(edge_index[1], np.int64)
    loops = np.arange(N, dtype=np.int64)
    src = np.concatenate([src, loops])
    dst = np.concatenate([dst, loops])

    order = np.argsort(dst, kind='stable')
    src_s, dst_s = src[order], dst[order]
    bounds = np.searchsorted(dst_s, np.arange(N + 1))

    s_n = src_s % NSHARD
    s_clsA = s_n < NHALF0                      # class of each edge (by src)

    # per-node class degrees (for position sorting)
    dA = np.bincount(dst_s[s_clsA], minlength=N)
    dB = np.bincount(dst_s[~s_clsA], minlength=N)

    # ---- pass 1: position maps.  Nodes are re-assigned to cores so that
    # all 8 cores see near-identical per-chunk degree profiles: per class,
    # sort ALL nodes of that class by (dA, dB) snake order, then deal
    # consecutive 128-node chunks round-robin to the cores. ----
    posmap = np.empty(N, np.int64)       # node -> position on its core
    coremap = np.empty(N, np.int64)      # node -> owning core
    for half in range(2):
        cls_nodes = np.concatenate([
            np.arange(c_ * NSHARD + half * NHALF0,
                      min(c_ * NSHARD + half * NHALF0 + NHALF0,
                          (c_ + 1) * NSHARD))
            for c_ in range(NCORES)])
        ordh = _snake_order(dA[cls_nodes], dB[cls_nodes], block=4096)
        snodes = cls_nodes[ordh]
        # deal: global chunk g (128 slots incl per-core dummies) ->
        # core g % 8, local chunk g // 8.  Slot 0 of each core's chunk 0
        # is the dummy; account by dealing positions 1.. within each core.
        for c_ in range(NCORES):
            share = snodes[c_::NCORES]
            posmap[share] = half * PHALF + 1 + np.arange(len(share))
            coremap[share] = c_

    # src table row: core * PHALF + (position within half)
    s_core = coremap[src_s]
    s_row = s_core * PHALF + (posmap[src_s] % PHALF)

    # ---- pass 2: per-core per-chunk max degree, shared across cores ----
    CA = np.zeros((NCORES, NCHUNK), np.int64)
    CB = np.zeros((NCORES, NCHUNK), np.int64)
    node_pos = posmap  # global node -> local position
    degA_all = np.bincount(dst_s[s_clsA], minlength=N)
    degB_all = np.bincount(dst_s[~s_clsA], minlength=N)
    for c_ in range(NCORES):
        nodes = np.where(coremap == c_)[0]
        ch = node_pos[nodes] // 128
        np.maximum.at(CA[c_], ch, degA_all[nodes])
        np.maximum.at(CB[c_], ch, degB_all[nodes])
    CAs = np.maximum(CA.max(axis=0), 1).astype(np.int64)
    CBs = np.maximum(CB.max(axis=0), 1).astype(np.int64)

    # ---- pass 3: per-core idx arrays ----
    def wrap(arr):  # [M] int16 -> [128, M//16] (16-part wrap, replicated x8)
        m = arr.shape[0]
        w = arr.reshape(m // 16, 16).T
        return np.tile(w, (8, 1))

    idx_cores = []
    for c_ in range(NCORES):
        pad_row = c_ * PHALF  # dummy position row (alpha_s = -60000)
        strips = []
        for cls, Cs in ((0, CAs), (1, CBs)):
            mats = [np.full((Cs[k], 128), pad_row, np.int64)
                    for k in range(NCHUNK)]
            strips.append(mats)
        for g in np.where(coremap == c_)[0]:
            sl = slice(bounds[g], bounds[g + 1])
            rows = s_row[sl]
            cls = s_clsA[sl]
            pos = node_pos[g]
            k, p = pos // 128, pos % 128
            ra = rows[cls]
            rb = rows[~cls]
            strips[0][k][0:len(ra), p] = ra
            strips[1][k][0:len(rb), p] = rb
        parts = []
        for cls in range(2):
            for k in range(NCHUNK):
                parts.append(wrap(strips[cls][k].reshape(-1)))
        idx_cores.append(np.concatenate(parts, axis=1).astype(np.int16))

    # strip offsets in int16 units within idx row
    offs = []
    o = 0
    for Cs in (CAs, CBs):
        oo = []
        for k in range(NCHUNK):
            oo.append(o)
            o += 8 * int(Cs[k])
        offs.append(oo)

    batch = np.asarray(batch, np.int64)
    cnt = np.bincount(batch, minlength=G).astype(np.float32)

    p01s = []
    for c_ in range(NCORES):
        nodes = np.where(coremap == c_)[0]
        pos = node_pos[nodes]
        p = np.zeros((NCHUNK, 128, G), np.float16)
        p[pos // 128, pos % 128, batch[nodes]] = 1.0
        p01s.append(p)

    grpA = _make_groups(CAs.tolist())
    grpB = _make_groups(CBs.tolist())

    pos_of_node = node_pos
    return (idx_cores, p01s, cnt, CAs.tolist(), CBs.tolist(),
            offs[0], offs[1], o, pos_of_node, coremap, grpA, grpB)


# ================= bass program =================

def _build_bass(CA, CB, offA, offB, idxw, grpA, grpB):
    import concourse.bass as bass
    import concourse.mybir as mybir
    import concourse.tile as tile
    from concourse import bacc
    from concourse.masks import make_identity

    fp16 = mybir.dt.float16
    fp32 = mybir.dt.float32
    i16 = mybir.dt.int16
    AF = mybir.ActivationFunctionType
    ALU = mybir.AluOpType
    AX = mybir.AxisListType

    KLAYERS = int(os.environ.get('KLAYERS', '3'))
    KCOLL = os.environ.get('KCOLL', '1') == '1'
    KEDGE = int(os.environ.get('KEDGE', '9'))
    MMAX = max(kk * cg for _, kk, cg in grpA + grpB)
    KKMAX = max(kk for _, kk, _ in grpA + grpB)

    nc = bacc.Bacc(num_swdge_queues=4)

    # ---- I/O ----
    xT = nc.dram_tensor("xT", [F_IN, NSP], fp16, kind="ExternalInput")
    idxs = nc.dram_tensor("idxs", [128, idxw], i16, kind="ExternalInput")
    p01 = nc.dram_tensor("p01", [NCHUNK, 128, G], fp16, kind="ExternalInput")
    Ws = nc.dram_tensor("Ws", [4, 128, HC], fp16, kind="ExternalInput")
    As = nc.dram_tensor("As", [6, HC, H], fp16, kind="ExternalInput")
    brep = nc.dram_tensor("brep", [3, 128, HC], fp32, kind="ExternalInput")
    out_d = nc.dram_tensor("out", [G, HC], fp32, kind="ExternalOutput")

    # ---- internal DRAM ----
    tbl_shard = nc.dram_tensor("tbl_shard", [NSP, 128], fp16)
    tblA = nc.dram_tensor("tblA", [HTBL, 128], fp16, addr_space="Shared")
    tblB = nc.dram_tensor("tblB", [HTBL, 128], fp16, addr_space="Shared")

    with tile.TileContext(nc) as tc:
        with (
            tc.tile_pool(name="const", bufs=1) as constp,
            tc.tile_pool(name="pers", bufs=1) as pers,
            tc.tile_pool(name="work", bufs=1) as work,
            tc.tile_pool(name="edge", bufs=4) as edgep,
            tc.tile_pool(name="edges", bufs=6) as edgesm,
            tc.tile_pool(name="edgetn", bufs=2) as edgetn,
            tc.tile_pool(name="dps", bufs=2, space="PSUM") as psd,
        ):
            cvals = set()
            for cc in set(CA) | set(CB):
                if cc <= 24:
                    cvals.add(cc)
                else:
                    cvals.add(cc // 2)
                    cvals.add(cc - cc // 2)
            regs = {cc: nc.gpsimd.to_reg(128 * cc) for cc in sorted(cvals)}
            expb = constp.tile([128, 1], fp32)
            nc.vector.memset(expb[:], EXP_SHIFT)
            negc = constp.tile([128, 1], fp16)
            nc.vector.memset(negc[:], MASK_NEG)
            ident = constp.tile([128, 128], fp16)
            make_identity(nc, ident[:])
            ws_t = constp.tile([128, 4 * HC], fp16)
            nc.sync.dma_start(ws_t[:].rearrange("p (a h) -> p a h", a=4),
                              Ws[:].rearrange("a p h -> p a h"))
            as_t = constp.tile([HC, 6 * H], fp16)
            nc.sync.dma_start(as_t[:].rearrange("c (s h) -> c s h", s=6),
                              As[:].rearrange("s c h -> c s h"))
            brep_t = constp.tile([128, 3 * HC], fp32)
            nc.sync.dma_start(brep_t[:].rearrange("p (l h) -> p l h", l=3),
                              brep[:].rearrange("l p h -> p l h"))

            act_fm = pers.tile([128, NSP], fp16)
            act_fmb = pers.tile([F_IN - 128, NSP], fp16)
            nc.sync.dma_start(act_fm[:], xT[0:128, :])
            nc.sync.dma_start(act_fmb[:], xT[128:F_IN, :])
            act_nm = pers.tile([128, NCHUNK, HC], fp16)
            adpos = pers.tile([128, NCHUNK, H], fp16)
            cmp = pers.tile([128, NCHUNK, 72], fp32)

            qrr = [0]
            gcount = [0]
            KTRED = os.environ.get('KTRED', '0') == '1'

            def reduce_kc(out_ap, in_kcf):
                """out[p,k,f] = sum_c in[p,k,c,f] via HW transpose-reduce
                (contiguous read), falling back to the strided rearrange."""
                ve = nc.vector
                if not KTRED:
                    p, kk, cg, f = in_kcf.shape
                    return ve.tensor_reduce(
                        out_ap, in_kcf.rearrange("p k c f -> p k f c"),
                        axis=AX.X, op=ALU.add)
                return ve.add_instruction(
                    mybir.InstTensorReduce(
                        name=f"I-{ve.bass.next_id()}",
                        op=ALU.add, axis=AX.X,
                        ins=[ve.lower_ap(in_kcf, opt=False)],
                        outs=[ve.lower_ap(out_ap)],
                        apply_absolute_value=None,
                        apply_transpose=True,
                        negate=None))

            def emit_dense(layer, c5lo, c5hi, trbuf):
                for ch in range(c5lo, c5hi):
                    cs512 = slice(ch * 512, (ch + 1) * 512)
                    hp = psd.tile([HC, 512], fp32, tag="dps")
                    if layer == 0:
                        nc.tensor.matmul(hp[:], ws_t[:, 0:HC],
                                         act_fm[0:128, cs512],
                                         start=True, stop=False)
                        nc.tensor.matmul(hp[:], ws_t[0:F_IN - 128, HC:2 * HC],
                                         act_fmb[:, cs512],
                                         start=False, stop=True)
                    else:
                        nc.tensor.matmul(
                            hp[:], ws_t[0:64, (layer + 1) * HC:(layer + 2) * HC],
                            act_fm[0:64, cs512], start=True, stop=True)
                    nc.scalar.activation(trbuf[0:64, cs512], hp[:], AF.Copy)
                    ap_ = psd.tile([40, 512], fp32, tag="dps")
                    nc.tensor.matmul(
                        ap_[0:H, :], as_t[:, 2 * layer * H:(2 * layer + 1) * H],
                        trbuf[0:64, cs512], start=True, stop=True)
                    nc.tensor.matmul(
                        ap_[32:32 + H, :],
                        as_t[:, (2 * layer + 1) * H:(2 * layer + 2) * H],
                        trbuf[0:64, cs512], start=True, stop=True,
                        tile_position=(0, 32))
                    nc.scalar.activation(trbuf[64:72, cs512], ap_[0:H, :],
                                         AF.Copy)
                    nc.scalar.activation(trbuf[96:96 + H, cs512],
                                         ap_[32:32 + H, :], AF.Copy)
                if c5lo == 0:
                    # dummy/pad rows: alpha_src = -60000 -> ex == 0
                    nc.vector.memset(trbuf[64:72, 0:1], MASK_NEG)
                    nc.vector.memset(trbuf[64:72, PHALF:PHALF + 1], MASK_NEG)

            def emit_table(klo, khi, trbuf):
                for ch in range(klo, khi):
                    tp = psd.tile([128, 104], fp16, tag="dps")
                    nc.tensor.matmul(tp[:], trbuf[0:104, ch * 128:(ch + 1) * 128],
                                     ident[0:104, 0:104], is_transpose=True,
                                     start=True, stop=True)
                    tabst = work.tile([128, 104], fp16, tag="tabst")
                    nc.scalar.activation(tabst[:], tp[:], AF.Copy)
                    nc.scalar.activation(adpos[:, ch, :], tabst[:, 96:104],
                                         AF.Copy)
                    # cols 104:128 of each table row stay garbage -- consumers
                    # only read cols 0:72.  Scalar-engine DMA queue: keeps
                    # these (dep-pending) writes off the sync ring that the
                    # edge-phase idx loads ride on.
                    nc.scalar.dma_start(
                        tbl_shard[ch * 128:(ch + 1) * 128, 0:104], tabst[:])

            def emit_ag(half):
                if not KCOLL:
                    return
                if half == 0:
                    nc.gpsimd.collective_compute(
                        "AllGather", mybir.AluOpType.bypass,
                        ins=[tbl_shard[0:PHALF, :]], outs=[tblA[:]],
                        replica_groups=[list(range(NCORES))])
                else:
                    nc.gpsimd.collective_compute(
                        "AllGather", mybir.AluOpType.bypass,
                        ins=[tbl_shard[PHALF:NSP, :]], outs=[tblB[:]],
                        replica_groups=[list(range(NCORES))])

            def emit_group(layer, phase, CL, offL, tbl, k0, KK, Cg):
                M = KK * Cg
                g1t = edgep.tile([128, MMAX, 128], fp16, tag="g1")
                if layer == 0 and gcount[0] < 4:
                    # first rotations: clear whatever (possibly NaN) bytes
                    # SBUF held; alpha_s = -60000 so unwritten cols -> ex == 0
                    nc.vector.memset(g1t[:], MASK_NEG)
                gcount[0] += 1
                o0 = offL[k0]
                ow = offL[k0 + KK - 1] + 8 * CL[k0 + KK - 1] - o0
                idxt = edgesm.tile([128, 8 * MMAX], i16, tag="idx")
                nc.sync.dma_start(idxt[:, 0:ow], idxs[:, o0:o0 + ow])
                for j in range(KK):
                    k = k0 + j
                    Ck = CL[k]
                    # split big chunks across two queues to break up
                    # long single-queue drain poles
                    parts = ((0, Ck),) if Ck <= 24 else \
                        ((0, Ck // 2), (Ck // 2, Ck - Ck // 2))
                    for (cb, cn) in parts:
                        nc.gpsimd.dma_gather(
                            g1t[:, j * Cg + cb:j * Cg + cb + cn, :], tbl[:],
                            idxt[:, offL[k] - o0 + 8 * cb:
                                 offL[k] - o0 + 8 * (cb + cn)],
                            num_idxs=128 * cn, num_idxs_reg=regs[cn],
                            elem_size=128, single_packet=False,
                            queue_num=qrr[0] % 4)
                        qrr[0] += 1
                zt = edgesm.tile([128, MMAX, H], fp16, tag="z")
                nc.vector.tensor_tensor(
                    zt[:, 0:M, :].rearrange("p (k c) h -> p k c h", k=KK),
                    g1t[:, 0:M, 64:72].rearrange("p (k c) h -> p k c h", k=KK),
                    adpos[:, k0:k0 + KK, :].unsqueeze(2)
                    .to_broadcast([128, KK, Cg, H]),
                    op=ALU.add)
                nc.vector.scalar_tensor_tensor(
                    zt[:, 0:M, :], zt[:, 0:M, :], NEG_SLOPE,
                    zt[:, 0:M, :], op0=ALU.mult, op1=ALU.max)
                nc.scalar.activation(g1t[:, 0:M, 64:72], zt[:, 0:M, :],
                                     AF.Exp, bias=expb[:], scale=1.0)
                nc.vector.tensor_tensor(
                    g1t[:, 0:M, 0:64].rearrange("p c (h j) -> p c h j", h=H),
                    g1t[:, 0:M, 0:64].rearrange("p c (h j) -> p c h j", h=H),
                    g1t[:, 0:M, 64:72].unsqueeze(3)
                    .to_broadcast([128, M, H, C]),
                    op=ALU.mult)
                if phase == 0:
                    reduce_kc(cmp[:, k0:k0 + KK, :],
                              g1t[:, 0:M, 0:72].rearrange(
                                  "p (k c) f -> p k c f", k=KK))
                else:
                    tn = edgetn.tile([128, KKMAX, 72], fp32, tag="tn")
                    reduce_kc(tn[:, 0:KK, :],
                              g1t[:, 0:M, 0:72].rearrange(
                                  "p (k c) f -> p k c f", k=KK))
                    nc.vector.tensor_tensor(
                        cmp[:, k0:k0 + KK, :], cmp[:, k0:k0 + KK, :],
                        tn[:, 0:KK, :], op=ALU.add)
                nc.scalar.activation(
                    g1t[:, :, 64:72],
                    negc[:].unsqueeze(2).to_broadcast([128, MMAX, H]),
                    AF.Copy)

            def emit_epilogue(layer, klo, khi):
                nk = khi - klo
                rs = work.tile([128, 32, H], fp32, tag="rs")
                nc.vector.tensor_scalar_max(cmp[:, klo:khi, 64:72],
                                            cmp[:, klo:khi, 64:72], 1e-30)
                nc.vector.reciprocal(rs[:, 0:nk, :], cmp[:, klo:khi, 64:72])
                ov = work.tile([128, 32, HC], fp32, tag="ov")
                nc.vector.tensor_tensor(
                    ov[:, 0:nk, :].rearrange("p c (h j) -> p c h j", h=H),
                    cmp[:, klo:khi, 0:64].rearrange(
                        "p c (h j) -> p c h j", h=H),
                    rs[:, 0:nk, :].unsqueeze(3)
                    .to_broadcast([128, nk, H, C]),
                    op=ALU.mult)
                nc.vector.tensor_tensor(
                    ov[:, 0:nk, :], ov[:, 0:nk, :],
                    (brep_t[:, layer * HC:(layer + 1) * HC]
                     .unsqueeze(1).to_broadcast([128, nk, HC])),
                    op=ALU.add)
                mneg = cmp[:, klo:khi, 0:64]  # num consumed; reuse as scratch
                nc.vector.tensor_scalar_min(mneg, ov[:, 0:nk, :], 0.0)
                nc.scalar.activation(mneg, mneg, AF.Exp)
                nc.vector.tensor_scalar_max(ov[:, 0:nk, :], ov[:, 0:nk, :], 0.0)
                nc.vector.scalar_tensor_tensor(
                    ov[:, 0:nk, :], mneg, -1.0, ov[:, 0:nk, :],
                    op0=ALU.add, op1=ALU.add)
                nc.scalar.activation(act_nm[:, klo:khi, :], ov[:, 0:nk, :],
                                     AF.Copy)

            def emit_transposes(klo, khi):
                for ch in range(klo, khi):
                    tp2 = psd.tile([64, 128], fp16, tag="dps")
                    nc.tensor.matmul(tp2[:], act_nm[:, ch, :],
                                     ident[:, 0:128], is_transpose=True,
                                     start=True, stop=True)
                    nc.scalar.activation(
                        act_fm[0:64, ch * 128:(ch + 1) * 128], tp2[:],
                        AF.Copy)

            SPL = 28  # == SPLIT_K (group lists never span it)
            DSPL = SPL * 128 // 512  # dense 512-chunk at the split

            # layer 0 prologue: dense + table + AGs
            trbuf = work.tile([128, NSP], fp16, tag="trbuf")
            emit_dense(0, 0, NSP // 512, trbuf)
            emit_table(0, PHALF // 128, trbuf)
            emit_ag(0)
            emit_table(PHALF // 128, NCHUNK, trbuf)
            emit_ag(1)

            for layer in range(KLAYERS):
                last = layer == KLAYERS - 1
                for (k0, KK, Cg) in grpA:
                    emit_group(layer, 0, CA, offA, tblA, k0, KK, Cg)
                for (k0, KK, Cg) in grpB:
                    if k0 >= SPL:
                        break
                    emit_group(layer, 1, CB, offB, tblB, k0, KK, Cg)
                emit_epilogue(layer, 0, SPL)
                if not last:
                    emit_transposes(0, SPL)
                    trbuf = work.tile([128, NSP], fp16, tag="trbuf")
                    emit_dense(layer + 1, 0, DSPL, trbuf)
                    emit_table(0, PHALF // 128, trbuf)
                for (k0, KK, Cg) in grpB:
                    if k0 < SPL:
                        continue
                    emit_group(layer, 1, CB, offB, tblB, k0, KK, Cg)
                if not last:
                    emit_ag(0)
                emit_epilogue(layer, SPL, NCHUNK)
                if not last:
                    emit_transposes(SPL, NCHUNK)
                    emit_dense(layer + 1, DSPL, NSP // 512, trbuf)
                    emit_table(PHALF // 128, NCHUNK, trbuf)
                    emit_ag(1)

            # ---------- pooling ----------
            p01t = work.tile([128, NCHUNK, G], fp16, tag="trbuf")
            nc.sync.dma_start(p01t[:], p01[:].rearrange("c p g -> p c g"))
            poolp = psd.tile([G, HC], fp32, tag="dps")
            for ch in range(NCHUNK):
                nc.tensor.matmul(poolp[:], p01t[:, ch, :], act_nm[:, ch, :],
                                 start=(ch == 0), stop=(ch == NCHUNK - 1))
            outsb = pers.tile([G, HC], fp32)
            nc.vector.tensor_copy(outsb[:], poolp[:])
            nc.sync.dma_start(out_d[:], outsb[:])

    nc.finalize()
    return nc


# ================= entry point =================

def _host_preprocess_cached(edge_index, batch):
    """Cache the (slow, pure-function-of-inputs) host preprocessing."""
    import hashlib
    import pickle
    key = hashlib.sha256()
    key.update(edge_index.tobytes())
    key.update(batch.tobytes())
    key.update(f"v12csr:{N}:{E}:{NSP}".encode())
    path = f"/tmp/gat_pre_{key.hexdigest()[:16]}.pkl"
    try:
        with open(path, "rb") as f:
            return pickle.load(f)
    except Exception:
        pass
    res = _host_preprocess(edge_index, batch)
    try:
        with open(path + ".tmp", "wb") as f:
            pickle.dump(res, f, protocol=4)
        os.replace(path + ".tmp", path)
    except Exception:
        pass
    return res


def kernel(x, edge_index, batch, W1, a1s, a1d, b1, W2, a2s, a2d, b2,
           W3, a3s, a3d, b3, Wlin, blin):
    x = np.asarray(x, np.float32)
    (idx_cores, p01s, cnt, CA, CB, offA, offB, idxw,
     pos_of_node, coremap, grpA, grpB) = _host_preprocess_cached(
        np.asarray(edge_index), np.asarray(batch))

    def amat(a):  # [H, C] -> [HC, H] block-diagonal
        m = np.zeros((HC, H), np.float16)
        a = np.asarray(a, np.float16)
        for h_ in range(H):
            m[h_ * C:(h_ + 1) * C, h_] = a[h_]
        return m

    Ws = np.zeros((4, 128, HC), np.float16)
    Ws[0] = np.asarray(W1, np.float16)[0:128]
    Ws[1, 0:F_IN - 128] = np.asarray(W1, np.float16)[128:F_IN]
    Ws[2, 0:HC] = np.asarray(W2, np.float16)
    Ws[3, 0:HC] = np.asarray(W3, np.float16)
    As = np.stack([amat(a1s), amat(a1d), amat(a2s), amat(a2d),
                   amat(a3s), amat(a3d)])
    brep = np.stack([np.tile(np.asarray(b, np.float32)[None, :], (128, 1))
                     for b in (b1, b2, b3)])

    in_maps = []
    for c_ in range(NCORES):
        nodes = np.where(coremap == c_)[0]
        pos = pos_of_node[nodes]
        xTa = np.zeros((F_IN, NSP), np.float16)
        xTa[:, pos] = x[nodes].T
        in_maps.append({
            "xT": xTa, "idxs": idx_cores[c_], "p01": p01s[c_],
            "Ws": Ws, "As": As, "brep": brep,
        })

    nc = _build_bass(CA, CB, offA, offB, idxw, grpA, grpB)
    from concourse.bass_utils import run_bass_kernel_spmd
    res = run_bass_kernel_spmd(nc, in_maps, list(range(NCORES)))
    global LAST_RESULT
    LAST_RESULT = res

    pooled = np.zeros((G, HC), np.float64)
    for r in res.results:
        pooled += r["out"].astype(np.float64)
    pooled = (pooled / np.maximum(cnt, 1.0)[:, None]).astype(np.float32)
    logits = (pooled @ np.asarray(Wlin, np.float32)
              + np.asarray(blin, np.float32))
    m = logits.max(axis=1, keepdims=True)
    lse = np.log(np.exp(logits - m).sum(axis=1, keepdims=True)) + m
    return (logits - lse).astype(np.float32)
